# revision 3
# baseline (speedup 1.0000x reference)
"""nn_Decoder on 8 trn2 NeuronCores.

Data-parallel over batch (8 shards of 8 across the cores) — the recurrent
state, attention and postnet are batch-independent, so the split is exact.
The 800-step recurrence is driven as a host loop around one jitted per-step
pmap (async dispatch keeps all 8 cores pipelined); the prenet / input-gate
projection / value cache run in one "pre" pmap and the output projection +
postnet in one "post" pmap. This keeps every compiled module small (seconds
to compile) instead of one 800-step scan NEFF (which takes ~an hour under
neuronxcc).
"""

import numpy as np
import jax, jax.numpy as jnp
from jax import lax

B, T, S = 64, 800, 200
M, PRE, H, E, A, LF, PF, K = 80, 256, 1024, 512, 128, 32, 512, 5
BN_EPS = 1e-5
NC = 8
TCH = 25  # scan steps per jitted chunk


def _conv1d_same(x, w):
    return lax.conv_general_dilated(x, w, (1,), "SAME",
                                    dimension_numbers=("NCH", "OIH", "NCH"))


def _pre(enc, mels, W):
    Bs = mels.shape[0]
    mels_ws = jnp.concatenate([jnp.zeros((Bs, 1, M), mels.dtype), mels], 1)[:, :T]
    x = jax.nn.relu(mels_ws @ W["pre_w0"].T)
    x = jax.nn.relu(x @ W["pre_w1"].T)
    vcache = enc @ W["wv"].T                                  # [Bs,S,A]
    xg = x @ W["wih1"][:, :PRE].T + (W["bih1"] + W["bhh1"])   # [Bs,T,4H]
    return xg, vcache


def _steps(carry, xg_c, vcache, enc, enc_mask, W):
    """Run TCH steps. xg_c: [TCH,Bs,4H]."""
    wih1_c = W["wih1"][:, PRE:]

    def step(carry, xg_t):
        h1, c1, h2, c2, cum, ctx = carry
        g = xg_t + ctx @ wih1_c.T + h1 @ W["whh1"].T
        i, f, gg, o = jnp.split(g, 4, axis=-1)
        c1n = jax.nn.sigmoid(f) * c1 + jax.nn.sigmoid(i) * jnp.tanh(gg)
        h1n = jax.nn.sigmoid(o) * jnp.tanh(c1n)
        g2 = h1n @ W["wih2"].T + W["bih2"] + h2 @ W["whh2"].T + W["bhh2"]
        i2, f2, g2g, o2 = jnp.split(g2, 4, axis=-1)
        c2n = jax.nn.sigmoid(f2) * c2 + jax.nn.sigmoid(i2) * jnp.tanh(g2g)
        h2n = jax.nn.sigmoid(o2) * jnp.tanh(c2n)
        q = h2n @ W["wq"].T + W["bq"]
        loc = _conv1d_same(cum[:, None, :], W["loc_w"])
        loc = jnp.einsum("bfs,af->bsa", loc, W["loc_proj"])
        e = jnp.tanh(q[:, None, :] + vcache + loc) @ W["vvec"]
        e = jnp.where(enc_mask, -1e9, e)
        aw = jax.nn.softmax(e, axis=1)
        ctxn = jnp.einsum("bs,bse->be", aw, enc)
        lin = jnp.concatenate([h2n, ctxn], axis=-1)
        mel = lin @ W["wf"].T + W["bf"]
        stp = (lin @ W["ws"].T + W["bs"])[:, 0]
        return (h1n, c1n, h2n, c2n, cum + aw, ctxn), (mel, stp, aw)

    carry, (mel, stp, aw) = lax.scan(step, carry, xg_c)
    return carry, mel, stp, aw


def _post(mel_outs, dec_mask, W):
    y = mel_outs.transpose(0, 2, 1)
    inv = np.float32(1.0 / np.sqrt(1.0 + BN_EPS))
    for i in range(5):
        y = _conv1d_same(y, W[f"pn_w{i}"]) + W[f"pn_b{i}"][None, :, None]
        y = y * inv * W[f"pn_g{i}"][None, :, None] + W[f"pn_bt{i}"][None, :, None]
        if i < 4:
            y = jnp.tanh(y)
    mel_res = y.transpose(0, 2, 1)
    return jnp.where(dec_mask[:, :, None], 0.0, mel_res)


_FNS = None


def _get_fns():
    global _FNS
    if _FNS is None:
        devs = jax.devices()[:NC]
        pre = jax.pmap(_pre, in_axes=(0, 0, None), devices=devs)
        steps = jax.pmap(_steps, in_axes=(0, 0, 0, 0, 0, None), devices=devs)
        post = jax.pmap(_post, in_axes=(0, 0, None), devices=devs)
        _FNS = (pre, steps, post)
    return _FNS


WKEYS = ["pre_w0", "pre_w1", "wih1", "whh1", "bih1", "bhh1", "wih2", "whh2",
         "bih2", "bhh2", "wq", "bq", "wv", "loc_w", "loc_proj", "vvec",
         "wf", "bf", "ws", "bs"]


def kernel(**inputs):
    W = {k: jnp.asarray(inputs[k], jnp.float32) for k in WKEYS}
    for i in range(5):
        for nm in ["pn_w", "pn_b", "pn_g", "pn_bt"]:
            W[f"{nm}{i}"] = jnp.asarray(inputs[nm][i], jnp.float32)

    Bs = B // NC
    shard = lambda x: np.ascontiguousarray(
        np.asarray(x).reshape((NC, Bs) + np.asarray(x).shape[1:]))
    enc = jnp.asarray(shard(np.asarray(inputs["encoder_outputs"], np.float32)))
    mels = jnp.asarray(shard(np.asarray(inputs["mels"], np.float32)))
    em = jnp.asarray(shard(np.asarray(inputs["encoder_mask"])))
    dm = jnp.asarray(shard(np.asarray(inputs["decoder_mask"])))

    pre, steps, post = _get_fns()
    xg, vcache = pre(enc, mels, W)          # [NC,Bs,T,4H], [NC,Bs,S,A]
    xg = xg.transpose(0, 2, 1, 3)           # [NC,T,Bs,4H]

    z = jnp.zeros((NC, Bs, H), jnp.float32)
    carry = (z, z, z, z, jnp.zeros((NC, Bs, S), jnp.float32),
             jnp.zeros((NC, Bs, E), jnp.float32))
    mels_o, stops, aws = [], [], []
    for t0 in range(0, T, TCH):
        carry, mel, stp, aw = steps(carry, xg[:, t0:t0 + TCH], vcache, enc, em, W)
        mels_o.append(mel); stops.append(stp); aws.append(aw)
    mel_outs = jnp.concatenate(mels_o, 1).transpose(0, 2, 1, 3)  # [NC,Bs,T,M]
    stop = jnp.concatenate(stops, 1).transpose(0, 2, 1)          # [NC,Bs,T]
    aw = jnp.concatenate(aws, 1).transpose(0, 2, 1, 3)           # [NC,Bs,T,S]

    mel_res = post(mel_outs, dm, W)
    aw = jnp.where(dm[:, :, :, None], 0.0, aw)
    stop = jnp.where(dm, 1000.0, stop)

    un = lambda x: np.asarray(x).reshape((B,) + x.shape[2:])
    return (un(mel_outs), un(mel_res), un(stop), un(aw))


# revision 7
# speedup vs baseline: 1.4722x; 1.4722x over previous
"""nn_Decoder on 8 trn2 NeuronCores.

Data-parallel over batch (8 shards of 8 across the cores) — the recurrent
state, attention and postnet are batch-independent, so the split is exact.
The 800-step recurrence is driven as a host loop around one jitted per-step
pmap (async dispatch keeps all 8 cores pipelined); the prenet / input-gate
projection / value cache run in one "pre" pmap and the output projection +
postnet in one "post" pmap. This keeps every compiled module small (seconds
to compile) instead of one 800-step scan NEFF (which takes ~an hour under
neuronxcc).
"""

import numpy as np
import jax, jax.numpy as jnp
from jax import lax

B, T, S = 64, 800, 200
M, PRE, H, E, A, LF, PF, K = 80, 256, 1024, 512, 128, 32, 512, 5
BN_EPS = 1e-5
NC = 8
TCH = 25  # scan steps per jitted chunk


def _conv1d_same(x, w):
    return lax.conv_general_dilated(x, w, (1,), "SAME",
                                    dimension_numbers=("NCH", "OIH", "NCH"))


def _pre(enc, mels, W):
    Bs = mels.shape[0]
    mels_ws = jnp.concatenate([jnp.zeros((Bs, 1, M), mels.dtype), mels], 1)[:, :T]
    x = jax.nn.relu(mels_ws @ W["pre_w0"].T)
    x = jax.nn.relu(x @ W["pre_w1"].T)
    vcache = enc @ W["wv"].T                                  # [Bs,S,A]
    xg = x @ W["wih1"][:, :PRE].T + (W["bih1"] + W["bhh1"])   # [Bs,T,4H]
    return xg.transpose(1, 0, 2), vcache                      # [T,Bs,4H]


def _steps(carry, xg_c, vcache, enc, enc_mask, W):
    """Run TCH steps. xg_c: [TCH,Bs,4H]."""
    wih1_c = W["wih1"][:, PRE:]

    def step(carry, xg_t):
        h1, c1, h2, c2, cum, ctx = carry
        g = xg_t + ctx @ wih1_c.T + h1 @ W["whh1"].T
        i, f, gg, o = jnp.split(g, 4, axis=-1)
        c1n = jax.nn.sigmoid(f) * c1 + jax.nn.sigmoid(i) * jnp.tanh(gg)
        h1n = jax.nn.sigmoid(o) * jnp.tanh(c1n)
        g2 = h1n @ W["wih2"].T + W["bih2"] + h2 @ W["whh2"].T + W["bhh2"]
        i2, f2, g2g, o2 = jnp.split(g2, 4, axis=-1)
        c2n = jax.nn.sigmoid(f2) * c2 + jax.nn.sigmoid(i2) * jnp.tanh(g2g)
        h2n = jax.nn.sigmoid(o2) * jnp.tanh(c2n)
        q = h2n @ W["wq"].T + W["bq"]
        loc = _conv1d_same(cum[:, None, :], W["loc_w"])
        loc = jnp.einsum("bfs,af->bsa", loc, W["loc_proj"])
        e = jnp.tanh(q[:, None, :] + vcache + loc) @ W["vvec"]
        e = jnp.where(enc_mask, -1e9, e)
        aw = jax.nn.softmax(e, axis=1)
        ctxn = jnp.einsum("bs,bse->be", aw, enc)
        lin = jnp.concatenate([h2n, ctxn], axis=-1)
        mel = lin @ W["wf"].T + W["bf"]
        stp = (lin @ W["ws"].T + W["bs"])[:, 0]
        return (h1n, c1n, h2n, c2n, cum + aw, ctxn), (mel, stp, aw)

    carry, (mel, stp, aw) = lax.scan(step, carry, xg_c)
    return carry, mel, stp, aw


def _post(mel_outs, dec_mask, W):
    y = mel_outs.transpose(0, 2, 1)
    inv = np.float32(1.0 / np.sqrt(1.0 + BN_EPS))
    for i in range(5):
        y = _conv1d_same(y, W[f"pn_w{i}"]) + W[f"pn_b{i}"][None, :, None]
        y = y * inv * W[f"pn_g{i}"][None, :, None] + W[f"pn_bt{i}"][None, :, None]
        if i < 4:
            y = jnp.tanh(y)
    mel_res = y.transpose(0, 2, 1)
    return jnp.where(dec_mask[:, :, None], 0.0, mel_res)


_FNS = None


def _get_fns():
    global _FNS
    if _FNS is None:
        devs = jax.devices()[:NC]
        pre = jax.pmap(_pre, in_axes=(0, 0, 0), devices=devs)
        steps = jax.pmap(_steps, in_axes=(0, 0, 0, 0, 0, 0), devices=devs)
        post = jax.pmap(_post, in_axes=(0, 0, 0), devices=devs)
        _FNS = (pre, steps, post)
    return _FNS


WKEYS = ["pre_w0", "pre_w1", "wih1", "whh1", "bih1", "bhh1", "wih2", "whh2",
         "bih2", "bhh2", "wq", "bq", "wv", "loc_w", "loc_proj", "vvec",
         "wf", "bf", "ws", "bs"]


def kernel(**inputs):
    devs = jax.devices()[:NC]
    W = {k: np.asarray(inputs[k], np.float32) for k in WKEYS}
    for i in range(5):
        for nm in ["pn_w", "pn_b", "pn_g", "pn_bt"]:
            W[f"{nm}{i}"] = np.asarray(inputs[nm][i], np.float32)
    W = jax.device_put_replicated(W, devs)

    Bs = B // NC
    shard = lambda x: jax.device_put_sharded(
        list(np.asarray(x).reshape((NC, Bs) + np.asarray(x).shape[1:])), devs)
    enc = shard(np.asarray(inputs["encoder_outputs"], np.float32))
    mels = shard(np.asarray(inputs["mels"], np.float32))
    em = shard(np.asarray(inputs["encoder_mask"]))
    dm = shard(np.asarray(inputs["decoder_mask"]))

    pre, steps, post = _get_fns()
    xg, vcache = pre(enc, mels, W)          # [NC,T,Bs,4H], [NC,Bs,S,A]

    z = jnp.zeros((NC, Bs, H), jnp.float32)
    carry = (z, z, z, z, jnp.zeros((NC, Bs, S), jnp.float32),
             jnp.zeros((NC, Bs, E), jnp.float32))
    mels_o, stops, aws = [], [], []
    for t0 in range(0, T, TCH):
        carry, mel, stp, aw = steps(carry, xg[:, t0:t0 + TCH], vcache, enc, em, W)
        mels_o.append(mel); stops.append(stp); aws.append(aw)
    mel_outs = jnp.concatenate(mels_o, 1).transpose(0, 2, 1, 3)  # [NC,Bs,T,M]
    stop = jnp.concatenate(stops, 1).transpose(0, 2, 1)          # [NC,Bs,T]
    aw = jnp.concatenate(aws, 1).transpose(0, 2, 1, 3)           # [NC,Bs,T,S]

    mel_res = post(mel_outs, dm, W)
    aw = jnp.where(dm[:, :, :, None], 0.0, aw)
    stop = jnp.where(dm, 1000.0, stop)

    un = lambda x: np.asarray(x).reshape((B,) + x.shape[2:])
    return (un(mel_outs), un(mel_res), un(stop), un(aw))
